# revision 13
# baseline (speedup 1.0000x reference)
"""Dynamic per-sample 3x3 conv (attention-mixed kernel bank) on 8 TRN2 cores.

v2: packed fp16 layout (192-pitch rows, no col pads -> every DMA run >=512B),
descending-unit conv order with combined [128,384] PSUM drains:

  - x sample stored packed fp16 in SBUF: partitions 0-63 = x rows (flat,
    192 cols/row, 2 zero rows head + zero tail), partitions 64-127 = same
    shifted one row down (SBUF->SBUF DMA dup, contiguous fp16 runs).
  - per 2-row unit u (N=384): 3 accumulating fp16 matmuls (kx taps s=0,1,2),
    K=128 (ci x 2 row taps ky0/ky1), M=128 (cols 0-63 = Y_lo, 64-127 = Y_hi
    ky2 partial for unit u-1). Units processed DESCENDING (96..0) so unit
    u's fold can read unit u+1's staged Y_hi: one identity matmul adds
    STG[64:128, slot(u+1)] into Y_lo, then ONE [128,384] engine drain pulls
    final-lo (+dynamic bias) AND Y_hi-hi to the fp16 staging ring.
  - packed layout makes out cols 0/191 read wrapped neighbors: 8 small
    N=192 edge matmuls per sample (reusing sub-APs of the mixed LHS tiles)
    recompute both edge columns; staged ring values are overwritten before
    the batched fp16 HBM flush (host converts xout fp16 -> f32).
  - attention branch on device as v1: pooling via DVE accum_out passes over
    the packed fp16 image, tiny MLP on PE bank 7 + ACT Gelu/Tanh LUTs,
    DVE mixes the kernel bank into fp16 lhsT quadrant tiles.
"""
import os
import numpy as np

B, C, H, W = 16, 64, 192, 192
NCORES, BPC = 8, 2
XB = 384                        # x[0,0] flat col (2 zero rows head)
XCOLS = 37680                   # 384 + 192*192 + 432 zero tail
UNIT = 384
NU = 97                         # units 96..0; u=96 produces Y_hi only
NS = 24                         # staging ring slots (descending reuse window)
SG = 4608                       # strip cols (24 rows)
NK, MID = 4, 8

_CACHE = {}


def _build():
    SKIP = set(os.environ.get("KERNEL_SKIP", "").split(","))
    import concourse.bacc as bacc
    import concourse.mybir as mybir
    import concourse.tile as tile
    A = mybir.AluOpType
    AF = mybir.ActivationFunctionType
    F32 = mybir.dt.float32
    F16 = mybir.dt.float16

    nc = bacc.Bacc(trn_type="TRN2", num_swdge_queues=2)
    xin = nc.dram_tensor("xin", [BPC, C, H, W], F16, kind="ExternalInput")
    wkt = nc.dram_tensor("wkt", [NK, 3, 128, 128], F32, kind="ExternalInput")
    w1t = nc.dram_tensor("w1t", [C, MID], F32, kind="ExternalInput")
    b1v = nc.dram_tensor("b1v", [MID, 1], F32, kind="ExternalInput")
    w2t = nc.dram_tensor("w2t", [MID, NK], F32, kind="ExternalInput")
    b2v = nc.dram_tensor("b2v", [NK, 1], F32, kind="ExternalInput")
    wbt = nc.dram_tensor("wbt", [NK, 128], F32, kind="ExternalInput")
    i4m = nc.dram_tensor("i4m", [NK, NK], F32, kind="ExternalInput")
    on4 = nc.dram_tensor("on4", [NK, 128], F32, kind="ExternalInput")
    i64 = nc.dram_tensor("i64", [128, 128], F16, kind="ExternalInput")
    xout = nc.dram_tensor("xout", [BPC, C, H, W], F16, kind="ExternalOutput")

    with tile.TileContext(nc) as tc:
        with tc.tile_pool(name="big", bufs=1) as big, \
             tc.tile_pool(name="med", bufs=1) as med, \
             tc.tile_pool(name="ps", bufs=1, space="PSUM") as psp:
            XPD = big.tile([128, 2, XCOLS], F16)      # double-buffered samples
            STG = med.tile([128, NS, UNIT], F16)      # drain ring (lo=out, hi=Yhi)
            EDGE = med.tile([128, 2, 2, 192], F16)    # [.,b,left/right,row]
            TRASH = med.tile([128, SG], F16)
            WKT = med.tile([128, 12, 128], F32)
            LHS = med.tile([128, 2, 3, 128], F16)     # mixed lhsT quadrants
            TMPA = med.tile([128, 128], F32)
            TMPB = med.tile([128, 128], F32)
            SM = med.tile([128, 64], F32)             # packed small constants
            PP = med.tile([128, 32], F32)             # pooling partials
            I64F = med.tile([128, 128], F16)
            WB4 = med.tile([NK, 128], F32)
            ON4 = med.tile([NK, 128], F32)
            POOLED = med.tile([128, 2], F32)
            HT = med.tile([MID, 2], F32)
            AT = med.tile([NK, 2], F32)
            ATS = med.tile([NK, 2], F32)
            DG = med.tile([NK, NK], F32)
            ATB = med.tile([128, 8], F32)
            BIASV = med.tile([128, 2], F32)           # lo=bdyn, hi=0
            P = psp.tile([128, 8, 512], F32)

            # ---- constant loads (gpsimd: keeps the SP sequencer free so
            # the first x strip load configures immediately) ----
            nc.gpsimd.dma_start(WKT[:, :, :], wkt[:].rearrange("n s p c -> p (n s) c"))
            nc.gpsimd.dma_start(SM[0:C, 0:MID], w1t[:])
            nc.gpsimd.dma_start(SM[0:MID, 8:9], b1v[:])
            nc.gpsimd.dma_start(SM[0:MID, 9:13], w2t[:])
            nc.gpsimd.dma_start(SM[0:NK, 13:14], b2v[:])
            nc.gpsimd.dma_start(SM[0:NK, 14:18], i4m[:])
            nc.gpsimd.dma_start(I64F[:], i64[:])
            nc.gpsimd.dma_start(WB4[:], wbt[:])
            nc.gpsimd.dma_start(ON4[:], on4[:])

            # ---- one-time zero head/tail (both sample buffers) ----
            for i in range(2):
                nc.vector.memset(XPD[:, i, 0:XB], 0.0)
                nc.vector.memset(XPD[:, i, XB + H * W:XCOLS], 0.0)
            nc.vector.memset(BIASV[C:128, :], 0.0)

            def phase_load_l(b):
                """Strip loads (ascending) on the sync/HWDGE queue: queue
                order = DMA priority."""
                for g in range(8):
                    c0 = XB + SG * g
                    nc.sync.dma_start(
                        XPD[0:C, b % 2, c0:c0 + SG],
                        xin[b, :, 24 * g:24 * g + 24, :])

            def phase_load_d(b, ghi, glo):
                """Row-shift dups, strips ghi..glo descending (conv order)."""
                for g in range(ghi, glo - 1, -1):
                    # dup dst spans chosen so dup_g reads only strip g (+head/
                    # tail zeros): dst [192+SG*g, 192+SG*(g+1)) clamped.
                    d0 = 191 if g == 0 else 192 + SG * g
                    d1 = 37441 if g == 7 else 192 + SG * (g + 1)
                    nc.sync.dma_start(XPD[C:128, b % 2, d0:d1],
                                      XPD[0:C, b % 2, d0 + 192:d1 + 192])

            def pool_chunk(b, g):
                c0 = XB + SG * g
                nc.vector.tensor_scalar(
                    out=TRASH[0:C, 0:SG], in0=XPD[0:C, b % 2, c0:c0 + SG],
                    scalar1=1.0, scalar2=0.0, op0=A.mult, op1=A.add,
                    accum_out=PP[0:C, 8 * b + g:8 * b + g + 1])

            def phase_attn(b):
                veng = nc.vector
                veng.tensor_reduce(
                    POOLED[0:C, b:b + 1], PP[0:C, 8 * b:8 * b + 8],
                    axis=mybir.AxisListType.X, op=A.add)
                nc.tensor.matmul(P[0:MID, 7, 400:401], SM[0:C, 0:MID],
                                 POOLED[0:C, b:b + 1], start=True, stop=True)
                nc.scalar.activation(HT[:, b:b + 1], P[0:MID, 7, 400:401],
                                     AF.Gelu, bias=SM[0:MID, 8:9], scale=1.0)
                nc.tensor.matmul(P[0:NK, 7, 402:403], SM[0:MID, 9:13],
                                 HT[:, b:b + 1], start=True, stop=True)
                # sigmoid(z) = 0.5*tanh(z/2) + 0.5 (host passes b2/2)
                nc.scalar.activation(AT[:, b:b + 1], P[0:NK, 7, 402:403],
                                     AF.Tanh, bias=SM[0:NK, 13:14], scale=0.5)
                veng.tensor_scalar(out=ATS[:, b:b + 1], in0=AT[:, b:b + 1],
                                   scalar1=0.5, scalar2=0.5,
                                   op0=A.mult, op1=A.add)
                # broadcast attn to 128 partitions: ones4x128^T @ diag(attn)
                veng.tensor_scalar(out=DG[:], in0=SM[0:NK, 14:18],
                                   scalar1=ATS[:, b:b + 1], scalar2=0.0,
                                   op0=A.mult, op1=A.add)
                nc.tensor.matmul(P[:, 7, 404:408], ON4[:], DG[:],
                                 start=True, stop=True)
                nc.scalar.copy(ATB[:, 4 * b:4 * b + 4], P[:, 7, 404:408])
                # dynamic bias bdyn on partitions 0:64 (wbt hi half zero)
                nc.tensor.matmul(P[:, 7, 408:409], WB4[:], ATS[:, b:b + 1],
                                 start=True, stop=True)
                nc.scalar.copy(BIASV[0:C, b:b + 1], P[0:C, 7, 408:409])

            def phase_mix(b):
                for s in range(3):
                    a0 = ATB[:, 4 * b + 0:4 * b + 1]
                    nc.vector.tensor_scalar(out=TMPA[:], in0=WKT[:, 0 * 3 + s, :],
                                            scalar1=a0, scalar2=0.0,
                                            op0=A.mult, op1=A.add)
                    nc.vector.scalar_tensor_tensor(
                        out=TMPB[:], in0=WKT[:, 1 * 3 + s, :],
                        scalar=ATB[:, 4 * b + 1:4 * b + 2], in1=TMPA[:],
                        op0=A.mult, op1=A.add)
                    nc.vector.scalar_tensor_tensor(
                        out=TMPA[:], in0=WKT[:, 2 * 3 + s, :],
                        scalar=ATB[:, 4 * b + 2:4 * b + 3], in1=TMPB[:],
                        op0=A.mult, op1=A.add)
                    nc.vector.scalar_tensor_tensor(
                        out=LHS[:, b, s, :], in0=WKT[:, 3 * 3 + s, :],
                        scalar=ATB[:, 4 * b + 3:4 * b + 4], in1=TMPA[:],
                        op0=A.mult, op1=A.add)

            def emit_edges(b):
                """Recompute out cols 0 and 191 (packed-layout wrap garbage).
                Left in P[0:64,7,0:192], right in P[0:64,7,192:384]."""
                if "edges" in SKIP:
                    return
                xb = XPD[:, b % 2, :]

                def col_ap(parts, start):
                    return xb[parts, start:start + 192 * 192].rearrange(
                        "p (r q) -> p r q", q=192)[:, :, 0:1].rearrange(
                        "p r q -> p (r q)")

                # left: taps kx in {1,2}; A: (ky0,ky1) via dup blocks, B: ky2
                for i, s in enumerate((1, 2)):
                    nc.tensor.matmul(P[0:C, 7, 0:192],
                                     LHS[0:128, b, s, 0:C],
                                     col_ap(slice(0, 128), 191 + s),
                                     start=(i == 0), stop=False,
                                     skip_group_check=True)
                for i, s in enumerate((1, 2)):
                    nc.tensor.matmul(P[0:C, 7, 0:192],
                                     LHS[0:C, b, s, C:128],
                                     col_ap(slice(0, C), XB + 192 + s - 1),
                                     start=False, stop=(i == 1),
                                     skip_group_check=True)
                # right: taps kx in {0,1}
                for i, s in enumerate((0, 1)):
                    nc.tensor.matmul(P[0:C, 7, 192:384],
                                     LHS[0:128, b, s, 0:C],
                                     col_ap(slice(0, 128), 382 + s),
                                     start=(i == 0), stop=False,
                                     skip_group_check=True)
                for i, s in enumerate((0, 1)):
                    nc.tensor.matmul(P[0:C, 7, 192:384],
                                     LHS[0:C, b, s, C:128],
                                     col_ap(slice(0, C), XB + 382 + s),
                                     start=False, stop=(i == 1),
                                     skip_group_check=True)
                nc.scalar.activation(
                    EDGE[0:C, b, :, :].rearrange("p e q -> p (e q)"),
                    P[0:C, 7, 0:384], AF.Identity,
                    bias=BIASV[0:C, b:b + 1], scale=1.0)

            def emit_unit(b, u):
                """Conv matmuls + early Y_hi drain, then deferred fold +
                final-lo drain for unit u+1 (keeps PE off the drain chain).
                Drains split ~60/40 ACT/DVE (DVE also pools/mixes)."""
                xb = XPD[:, b % 2, :]
                bank = u % 8
                for s in range(3):
                    nc.tensor.matmul(P[:, bank, 0:UNIT],
                                     LHS[:, b, s, :],
                                     xb[:, 191 + UNIT * u + s:
                                        191 + UNIT * u + s + UNIT],
                                     start=(s == 0), stop=(s == 2))
                if u > 0:   # unit 0's Y_hi is never consumed
                    if u % 5 < 3:
                        nc.scalar.activation(STG[C:128, u % NS, :],
                                             P[C:128, bank, 0:UNIT], AF.Identity)
                    else:
                        nc.vector.tensor_copy(STG[C:128, u % NS, :],
                                              P[C:128, bank, 0:UNIT])
                if u <= 94:
                    v = u + 1
                    nc.tensor.matmul(P[0:C, v % 8, 0:UNIT], I64F[C:128, 0:C],
                                     STG[C:128, (v + 1) % NS, :],
                                     start=False, stop=True,
                                     skip_group_check=True)
                    emit_lo_drain(b, v, 1)

            def emit_lo_drain(b, v, n):
                """Drain folded lo of units v..v+n-1 (adjacent banks/slots)
                + dynamic bias; ~60/40 ACT/DVE split."""
                src = P[0:C, v % 8:v % 8 + n, 0:UNIT]
                dst = STG[0:C, v % NS:v % NS + n, :]
                if (v // 2) % 5 < 3:
                    nc.scalar.activation(dst, src, AF.Identity,
                                         bias=BIASV[0:C, b:b + 1], scale=1.0)
                else:
                    nc.vector.tensor_scalar(
                        out=dst, in0=src, scalar1=BIASV[0:C, b:b + 1],
                        scalar2=0.0, op0=A.add, op1=A.add)

            def emit_tail(b):
                """Fold + lo drain for unit 0 (not covered by the loop)."""
                nc.tensor.matmul(P[0:C, 0, 0:UNIT], I64F[C:128, 0:C],
                                 STG[C:128, 1 % NS, :],
                                 start=False, stop=True, skip_group_check=True)
                emit_lo_drain(b, 0, 1)

            def emit_flush(b, g):
                """Accumulate Y_hi(8g+1..8g+8) into lo slots via SBUF accum
                DMA, overwrite edge cols, then flush rows 16g..16g+16."""
                sl0 = (8 * g) % NS
                if "ow" not in SKIP:
                    view = STG[0:C, sl0:sl0 + 8, :].rearrange(
                        "p s (r q) -> p s r q", q=192)
                    nc.vector.tensor_copy(
                        view[:, :, :, 0:1].rearrange("p s r q -> p s (r q)"),
                        EDGE[0:C, b, 0, 16 * g:16 * g + 16].rearrange(
                            "p (s r) -> p s r", r=2))
                    nc.vector.tensor_copy(
                        view[:, :, :, 191:192].rearrange("p s r q -> p s (r q)"),
                        EDGE[0:C, b, 1, 16 * g:16 * g + 16].rearrange(
                            "p (s r) -> p s r", r=2))
                nc.sync.dma_start(
                    xout[b, :, 16 * g:16 * g + 16, :],
                    STG[0:C, sl0:sl0 + 8, :].rearrange("p s c -> p (s c)"))

            # ================= schedule =================
            phase_load_l(0)
            phase_load_d(0, 7, 4)
            for g in range(8):
                pool_chunk(0, g)
            phase_attn(0)
            phase_mix(0)
            phase_load_l(1)          # b1 loads before b0's low dups: conv
            phase_load_d(0, 3, 0)    # consumes strips descending, so strips
            phase_load_d(1, 7, 0)    # 3..0 aren't needed until much later

            def conv_phase(b, first):
                for u in range(96, -1, -1):
                    emit_unit(b, u)
                    if first:
                        if u == 78:
                            # bank 7 free: lo(79) drained this iter, next
                            # bank-7 tenant is unit 71; dup DMA done
                            emit_edges(b)
                            emit_flush(b, 11)
                            emit_flush(b, 10)
                        if u % 8 == 7 and (u + 1) // 8 <= 9:
                            emit_flush(b, (u + 1) // 8)
                        # interleave b1 pooling once its strips have landed
                        if 53 <= u <= 74 and u % 3 == 2:
                            pool_chunk(1, (74 - u) // 3)
                        if u == 52:
                            phase_attn(1)
                        if u == 48:
                            phase_mix(1)
                    elif u % 8 == 7 and (u + 1) // 8 <= 11:
                        emit_flush(b, (u + 1) // 8)
                emit_tail(b)
                emit_flush(b, 0)

            conv_phase(0, True)
            emit_edges(1)
            conv_phase(1, False)
    nc.compile()
    return nc


def _prep_inputs(x, w1, b1, w2, b2, Wk, Wb):
    """Host-side layout prep (pure reshaping of constant inputs)."""
    xs = np.ascontiguousarray(x.reshape(NCORES, BPC, C, H, W)).astype(np.float16)
    wkT = np.zeros((NK, 3, 128, 128), np.float32)
    # Wk: [n, co, ci, ky, kx] -> lhsT quadrants [ci(+64*row-tap), co(+64*hi)]
    Wt = np.transpose(Wk, (0, 4, 3, 2, 1))  # [n, kx, ky, ci, co]
    wkT[:, :, 0:64, 0:64] = Wt[:, :, 0]      # ky=0 -> lo (K rows 0-63)
    wkT[:, :, 64:128, 0:64] = Wt[:, :, 1]    # ky=1 -> lo (K rows 64-127)
    wkT[:, :, 0:64, 64:128] = Wt[:, :, 2]    # ky=2 -> Y_hi (K rows 0-63)
    w1t = np.ascontiguousarray(w1.T).astype(np.float32) / float(H * W)
    w2t = np.ascontiguousarray(w2.T).astype(np.float32)
    wbt = np.zeros((NK, 128), np.float32)
    wbt[:, 0:64] = Wb
    i64 = np.zeros((128, 128), np.float16)
    i64[64:128, 0:64] = np.eye(64, dtype=np.float16)
    maps = []
    for core in range(NCORES):
        maps.append({
            "xin": xs[core],
            "wkt": wkT,
            "w1t": w1t,
            "b1v": b1.reshape(MID, 1).astype(np.float32),
            "w2t": w2t,
            "b2v": (b2.reshape(NK, 1) / 2.0).astype(np.float32),
            "wbt": wbt,
            "i4m": np.eye(NK, dtype=np.float32),
            "on4": np.ones((NK, 128), np.float32),
            "i64": i64,
        })
    return maps


def kernel(x, w1, b1, w2, b2, Wk, Wb):
    from concourse import bass_utils
    if "nc" not in _CACHE:
        _CACHE["nc"] = _build()
    nc = _CACHE["nc"]
    in_maps = _prep_inputs(np.asarray(x, np.float32), np.asarray(w1),
                           np.asarray(b1), np.asarray(w2), np.asarray(b2),
                           np.asarray(Wk), np.asarray(Wb))
    res = bass_utils.run_bass_kernel_spmd(
        nc, in_maps, core_ids=list(range(NCORES)),
        trace=bool(int(os.environ.get("KERNEL_TRACE", "0"))))
    _CACHE["last_result"] = res
    out = np.empty((B, C, H, W), np.float32)
    for core in range(NCORES):
        out[core * BPC:(core + 1) * BPC] = np.asarray(
            res.results[core]["xout"]).astype(np.float32)
    return out


def _patch_sim_gelu():
    import concourse.bass_interp as bi
    import concourse.mybir as mb
    from scipy.special import erf
    if getattr(bi.InstructionExecutor, "_gelu_patched", False):
        return
    orig = bi.InstructionExecutor.visit_InstActivation

    def patched(self, instruction, **kw):
        if instruction.func == mb.ActivationFunctionType.Gelu:
            instruction.func = mb.ActivationFunctionType.Identity
            try:
                r = orig(self, instruction, **kw)
            finally:
                instruction.func = mb.ActivationFunctionType.Gelu
            v = self.view_ap(instruction.outs[0], bi.Direction.WRITE, instruction)
            y = np.asarray(v, np.float64)
            v[:] = (y * 0.5 * (1.0 + erf(y / np.sqrt(2.0)))).astype(np.float32)
            return r
        return orig(self, instruction, **kw)

    bi.InstructionExecutor.visit_InstActivation = patched
    bi.InstructionExecutor._gelu_patched = True


def simulate_core0(x, w1, b1, w2, b2, Wk, Wb):
    """CoreSim path for numeric validation without hardware (core 0 only)."""
    from concourse.bass_interp import CoreSim
    _patch_sim_gelu()
    if "nc" not in _CACHE:
        _CACHE["nc"] = _build()
    nc = _CACHE["nc"]
    in_maps = _prep_inputs(np.asarray(x, np.float32), np.asarray(w1),
                           np.asarray(b1), np.asarray(w2), np.asarray(b2),
                           np.asarray(Wk), np.asarray(Wb))
    sim = CoreSim(nc)
    for k, v in in_maps[0].items():
        sim.tensor(k)[:] = v
    sim.simulate()
    return np.array(sim.tensor("xout")).astype(np.float32)
